# revision 18
# baseline (speedup 1.0000x reference)
"""Dilated attention (segment 64, dilation 4, 16 heads, head_dim 64) on 8 trn2 cores.

Sharding: 2 batches x 4 head-groups (4 heads each) = 8 cores. Each core computes
q/k/v projections for its 4 heads on its batch, block-sparse attention over the
+-2 block (256-token) dilated band, and a partial output projection. Host sums
the 4 head-group partials per batch (partials shipped bf16).

Layout is fully "transposed" on-core to avoid PE transposes:
  xT   [D, S]    (D on partitions, 8 chunks of 128), DMA'd in 4 S-quarters
  qT/kT [64h, S] per head (head dim on partitions), tiled in 4 chunks of 512
  v    [S, 64]   natural (keys on partitions) + ones column -> softmax denoms
  scoresT [k-block 128, q-window <=640] = kT_blk-stationary x qT-window
  exp (no max subtraction; scores are N(0,1)-ish), binary mask multiply,
  PV accumulates outT [65, 512-chunk] per head in rolling PSUM banks
  (row 64 = denominators), normalized per chunk as its last key-block lands,
  yT [D, S] = wo-stationary x outT_norm, staged bf16, summed on host.

Scheduling: emission order = Tile scheduler priority. The attention chain
(scores -> exp -> mask -> PV) is emitted first per pair; projection matmuls
(v, pair-1 q/k, y) are emitted after so the PE fills exp-paced dependency
stalls with them and the HAM clock gate never sees an idle window.
"""

import numpy as np
import ml_dtypes

bfloat16 = ml_dtypes.bfloat16

B, S, D = 2, 2048, 1024
H, Dh = 16, 64
NCORES = 8
NKB = S // 128  # 16 key blocks
WMAX = 640

_cache = {}


def _mask_rel():
    kp = np.arange(128)[:, None]
    j = np.arange(WMAX)[None, :]
    qrel = j - 256
    diff = np.abs(qrel - kp)
    seg = (qrel // 64) == (kp // 64)
    dil = (diff > 0) & (diff % 4 == 0) & (diff <= 256)
    return np.ascontiguousarray((seg | dil).astype(bfloat16))


def _win(kb):
    return max(0, kb - 2) * 128, min(NKB, kb + 3) * 128


def _pieces(q0, q1):
    """Split [q0,q1) at absolute 512 boundaries (psum bank + q-tile chunks)."""
    out = []
    a = q0
    while a < q1:
        b = min(q1, (a // 512 + 1) * 512)
        out.append((a, b))
        a = b
    return out


def _pieces_sc(q0, q1):
    """Split [q0,q1) at absolute 512 boundaries (q-tile chunks) AND at
    q0+512 (the [128,640] f32 score-psum tile's internal bank boundary)."""
    pts = {b for b in range((q0 // 512 + 1) * 512, q1, 512)}
    if q0 + 512 < q1:
        pts.add(q0 + 512)
    out = []
    a = q0
    for b in sorted(pts) + [q1]:
        if b > a:
            out.append((a, b))
            a = b
    return out


def _build(debug=False):
    key = ("nc", debug)
    if key in _cache:
        return _cache[key]
    import concourse.mybir as mybir
    from concourse import bacc
    from concourse.tile import TileContext

    bf = mybir.dt.bfloat16
    f32 = mybir.dt.float32
    EXP = mybir.ActivationFunctionType.Exp

    nc = bacc.Bacc()
    d_x = nc.declare_dram_parameter("xT", [128, 8, S], bf, isOutput=False)
    d_wq = nc.declare_dram_parameter("wq", [128, 8, 256], bf, isOutput=False)
    d_wk = nc.declare_dram_parameter("wk", [128, 8, 256], bf, isOutput=False)
    d_wv = nc.declare_dram_parameter("wv", [128, 8, 256], bf, isOutput=False)
    d_wo = nc.declare_dram_parameter("wo", [128, 2, 1024], bf, isOutput=False)
    d_mask = nc.declare_dram_parameter("maskT", [128, WMAX], bf, isOutput=False)
    d_y = nc.declare_dram_parameter("yT", [128, 8, S], bf, isOutput=True)
    if debug:
        d_dbg_q = nc.declare_dram_parameter("dbg_q", [128, S], bf, isOutput=True)
        d_dbg_k = nc.declare_dram_parameter("dbg_k", [128, S], bf, isOutput=True)
        d_dbg_v = nc.declare_dram_parameter("dbg_v", [128, 16 * 4 * 65], bf, isOutput=True)
        d_dbg_at = nc.declare_dram_parameter("dbg_at", [128, WMAX], bf, isOutput=True)
        d_dbg_on = nc.declare_dram_parameter("dbg_on", [128, S], bf, isOutput=True)
        d_dbg_den = nc.declare_dram_parameter("dbg_den", [4, S], f32, isOutput=True)
        d_dbg_rec = nc.declare_dram_parameter("dbg_rec", [4, S], f32, isOutput=True)

    with TileContext(nc) as tc:
        with (
            tc.tile_pool(name="const", bufs=1) as cpool,
            tc.tile_pool(name="attn", bufs=44) as apool,
            tc.tile_pool(name="ybig", bufs=2) as ypool,
            tc.tile_pool(name="small", bufs=4) as spool,
        ):
            # ---- input DMAs: weights+mask on scalar queue, x quarters split
            # across sync/gpsimd queues so the first quarter lands early ----
            sb_wq = cpool.tile([128, 8, 256], bf, name="wq", tag="wq")
            sb_wk = cpool.tile([128, 8, 256], bf, name="wk", tag="wk")
            nc.scalar.dma_start(out=sb_wq, in_=d_wq[:, :, :])
            nc.scalar.dma_start(out=sb_wk, in_=d_wk[:, :, :])
            sb_xq = []
            for sq in range(4):
                t = cpool.tile([128, 8, 512], bf, name=f"xq{sq}", tag=f"xq{sq}")
                nc.sync.dma_start(out=t[:, 0:4, :], in_=d_x[:, 0:4, sq * 512:(sq + 1) * 512])
                nc.gpsimd.dma_start(out=t[:, 4:8, :], in_=d_x[:, 4:8, sq * 512:(sq + 1) * 512])
                sb_xq.append(t)
            sb_mask = cpool.tile([128, WMAX], bf, name="mask", tag="mask")
            nc.scalar.dma_start(out=sb_mask, in_=d_mask[:, :])
            sb_wv = cpool.tile([128, 8, 256], bf, name="wv", tag="wv")
            nc.scalar.dma_start(out=sb_wv, in_=d_wv[:, :, :])
            sb_wo = cpool.tile([128, 2, 1024], bf, name="wo", tag="wo")
            nc.scalar.dma_start(out=sb_wo, in_=d_wo[:, :, :])

            sb_q = [[cpool.tile([128, 512], bf, name=f"q{p}{t}", tag=f"q{p}{t}")
                     for t in range(4)] for p in range(2)]
            sb_k = [[cpool.tile([128, 512], bf, name=f"k{p}{t}", tag=f"k{p}{t}")
                     for t in range(4)] for p in range(2)]
            sb_on = [[cpool.tile([128, 512], bf, name=f"on{p}{t}", tag=f"on{p}{t}")
                      for t in range(4)] for p in range(2)]
            sb_v = cpool.tile([128, 16, 4, 65], bf, name="v", tag="v")
            nc.vector.memset(sb_v[:, :, :, 64:65], 1.0)

            # ---- HAM warmup: junk matmuls lift the PE clock while DMAs land ----
            with tc.tile_pool(name="warm", bufs=1, space="PSUM") as wpool:
                junk = cpool.tile([128, 512], bf, name="junk", tag="junk")
                nc.vector.memset(junk, 0.0)
                wps = wpool.tile([128, 512], f32, name="wps", tag="wps")
                for i in range(14):
                    nc.tensor.matmul(wps, lhsT=junk[:, 0:128], rhs=junk,
                                     start=(i == 0), stop=(i == 13))

            # ---- main era: pj(2) + sc(4) + ot(2) psum banks ----
            with (
                tc.tile_pool(name="pj", bufs=2, space="PSUM") as pj,
                tc.tile_pool(name="sc", bufs=2, space="PSUM") as scp,
                tc.tile_pool(name="ot", bufs=2, space="PSUM") as otp,
            ):
                def proj_qk(w_sb, dst, p, tts, copy_eng, scope):
                    with nc.named_scope(scope):
                        for tt in tts:
                            ps = pj.tile([128, 512], f32, name="pspj", tag="pj")
                            for dc in range(8):
                                nc.tensor.matmul(
                                    ps,
                                    lhsT=w_sb[:, dc, p * 128:(p + 1) * 128],
                                    rhs=sb_xq[tt][:, dc, :],
                                    start=(dc == 0),
                                    stop=(dc == 7),
                                )
                            if copy_eng == "scalar":
                                nc.scalar.copy(dst[p][tt], ps)
                            else:
                                nc.vector.tensor_copy(dst[p][tt], ps)

                def proj_v(ts):
                    with nc.named_scope("proj_v"):
                        for t in ts:
                            ps = pj.tile([128, 256], f32, name="psv", tag="pj")
                            for dc in range(8):
                                nc.tensor.matmul(
                                    ps,
                                    lhsT=sb_xq[t // 4][:, dc, (t % 4) * 128:(t % 4) * 128 + 128],
                                    rhs=sb_wv[:, dc, :],
                                    start=(dc == 0),
                                    stop=(dc == 7),
                                )
                            src = ps.rearrange("p (h d) -> p h d", h=4)
                            if t % 2 == 0:
                                nc.scalar.copy(sb_v[:, t, :, 0:64], src)
                            else:
                                nc.vector.tensor_copy(sb_v[:, t, :, 0:64], src)

                def scores_kb(p, kb, ats):
                    q0, q1 = _win(kb)
                    wk_ = q1 - q0
                    j0 = q0 - (kb - 2) * 128
                    kbt, kbo = kb // 4, (kb % 4) * 128
                    for hh in range(2):
                        half = hh * 64
                        sc = scp.tile([128, WMAX], f32, name="sc", tag="sc")
                        for a, b in _pieces_sc(q0, q1):
                            nc.tensor.matmul(
                                sc[:, a - q0:b - q0],
                                lhsT=sb_k[p][kbt][half:half + 64, kbo:kbo + 128],
                                rhs=sb_q[p][a // 512][half:half + 64, a % 512:a % 512 + b - a],
                                start=True,
                                stop=True,
                            )
                        at = apool.tile([128, WMAX], bf, name="at", tag="at")
                        nc.scalar.activation(at[:, :wk_], sc[:, :wk_], EXP)
                        nc.vector.tensor_mul(
                            at[:, :wk_], at[:, :wk_], sb_mask[:, j0:j0 + wk_]
                        )
                        ats[p, hh, kb] = at
                        if debug and p == 0 and hh == 0 and kb == 8:
                            nc.sync.dma_start(out=d_dbg_at[:, :], in_=at[:, :])

                def pv_kb(p, hh, kb, ats, outc):
                    h = 2 * p + hh
                    half = hh * 64
                    q0, q1 = _win(kb)
                    at = ats[p, hh, kb]
                    vv = sb_v[:, kb, h, :]
                    for a, b in _pieces(q0, q1):
                        c = a // 512
                        if c not in outc:
                            outc[c] = otp.tile([65, 512], f32, name=f"o{h}{c}", tag="ot")
                        nc.tensor.matmul(
                            outc[c][:, a - 512 * c:b - 512 * c],
                            lhsT=vv,
                            rhs=at[:, a - q0:b - q0],
                            start=(kb == max(0, 4 * c - 2)),
                            stop=(kb == min(NKB - 1, 4 * c + 5)),
                        )
                    for c in sorted(outc):
                        if kb == min(NKB - 1, 4 * c + 5):
                            ot = outc.pop(c)
                            den = spool.tile([1, 512], f32, name="den", tag="den")
                            nc.vector.tensor_copy(den, ot[64:65, :])
                            rec = spool.tile([1, 512], f32, name="rec", tag="rec")
                            nc.vector.reciprocal_approx_fast(rec, den)
                            bc = spool.tile([64, 512], f32, name="bc", tag="bc")
                            nc.gpsimd.partition_broadcast(bc, rec)
                            if debug and p == 0:
                                nc.sync.dma_start(out=d_dbg_den[h:h + 1, c * 512:(c + 1) * 512], in_=den)
                                nc.sync.dma_start(out=d_dbg_rec[h:h + 1, c * 512:(c + 1) * 512], in_=rec)
                            nc.vector.tensor_mul(
                                sb_on[p][c][half:half + 64, :],
                                ot[0:64, :],
                                bc,
                            )

                def proj_y():
                    with nc.named_scope("proj_y"):
                        for tt in range(4):
                            yb = ypool.tile([128, 8, 512], bf, name=f"yb{tt}", tag="ybig")
                            for dc in range(8):
                                ps = pj.tile([128, 512], f32, name="psy", tag="pj")
                                nc.tensor.matmul(
                                    ps, lhsT=sb_wo[:, 0, dc * 128:(dc + 1) * 128],
                                    rhs=sb_on[0][tt], start=True, stop=False,
                                )
                                nc.tensor.matmul(
                                    ps, lhsT=sb_wo[:, 1, dc * 128:(dc + 1) * 128],
                                    rhs=sb_on[1][tt], start=False, stop=True,
                                )
                                if dc % 2 == 0:
                                    nc.scalar.copy(yb[:, dc, :], ps)
                                else:
                                    nc.vector.tensor_copy(yb[:, dc, :], ps)
                            eng = nc.sync if tt % 2 == 0 else nc.gpsimd
                            eng.dma_start(out=d_y[:, :, tt * 512:(tt + 1) * 512], in_=yb)

                # projections: filler priority, gated by x-quarter DMAs
                for sq in range(4):
                    proj_qk(sb_wq, sb_q, 0, [sq], "scalar", "proj_q0")
                    proj_qk(sb_wk, sb_k, 0, [sq], "scalar", "proj_k0")
                    proj_qk(sb_wq, sb_q, 1, [sq], "scalar", "proj_q1")
                    proj_qk(sb_wk, sb_k, 1, [sq], "scalar", "proj_k1")
                    proj_v(range(4 * sq, 4 * sq + 4))
                # Attention chains emitted FIRST (= highest scheduler priority):
                # both pairs interleaved per kb so the exp stream is continuous.
                # Projections are emitted after -- the PE prefers ready attention
                # matmuls and fills every stall with projection work.
                ats = {}
                with nc.named_scope("scores"):
                    for kb in range(NKB):
                        scores_kb(0, kb, ats)
                        scores_kb(1, kb, ats)
                for p, hh in ((0, 0), (1, 0), (0, 1), (1, 1)):
                    with nc.named_scope(f"pv_h{2 * p + hh}"):
                        outc = {}
                        for kb in range(NKB):
                            pv_kb(p, hh, kb, ats, outc)
                if debug:
                    for t in range(4):
                        nc.sync.dma_start(out=d_dbg_q[:, t * 512:(t + 1) * 512], in_=sb_q[0][t])
                        nc.sync.dma_start(out=d_dbg_k[:, t * 512:(t + 1) * 512], in_=sb_k[0][t])
                        nc.sync.dma_start(out=d_dbg_on[:, t * 512:(t + 1) * 512], in_=sb_on[0][t])
                    nc.sync.dma_start(
                        out=d_dbg_v[:, :],
                        in_=sb_v.rearrange("p a b c -> p (a b c)"),
                    )
                proj_y()

    nc.compile()
    _cache[key] = nc
    return nc


def kernel(hidden_states, w_q, w_k, w_v, w_o, _debug=False):
    from concourse.bass_utils import run_bass_kernel_spmd

    nc = _build(debug=_debug)
    mask = _mask_rel()
    scale = np.float32(Dh ** -0.5)

    def chunk_dmajor(w, rows, cols):
        return np.ascontiguousarray(
            w.reshape(rows, 128, cols).transpose(1, 0, 2)
        )

    in_maps = []
    for c in range(NCORES):
        b, hg = c // 4, c % 4
        hsl = slice(hg * 256, (hg + 1) * 256)
        xT = np.asarray(hidden_states[b]).T.astype(bfloat16)  # [D, S]
        in_maps.append({
            "xT": chunk_dmajor(xT, 8, S),
            "wq": chunk_dmajor((np.asarray(w_q[:, hsl]) * scale).astype(bfloat16), 8, 256),
            "wk": chunk_dmajor(np.asarray(w_k[:, hsl]).astype(bfloat16), 8, 256),
            "wv": chunk_dmajor(np.asarray(w_v[:, hsl]).astype(bfloat16), 8, 256),
            "wo": chunk_dmajor(np.asarray(w_o[hsl, :]).astype(bfloat16), 2, 1024),
            "maskT": mask,
        })

    res = run_bass_kernel_spmd(nc, in_maps, list(range(NCORES)))
    _cache["last_results"] = res

    y = np.zeros((B, S, D), np.float32)
    for c in range(NCORES):
        yT = np.asarray(res.results[c]["yT"], np.float32)  # [128, 8, S]
        y[c // 4] += yT.transpose(1, 0, 2).reshape(D, S).T
    return y


# revision 19
# speedup vs baseline: 1.0290x; 1.0290x over previous
"""Dilated attention (segment 64, dilation 4, 16 heads, head_dim 64) on 8 trn2 cores.

Sharding: 2 batches x 4 head-groups (4 heads each) = 8 cores. Each core computes
q/k/v projections for its 4 heads on its batch, block-sparse attention over the
+-2 block (256-token) dilated band, and a partial output projection. Host sums
the 4 head-group partials per batch (partials shipped bf16).

Layout is fully "transposed" on-core to avoid PE transposes:
  xT   [D, S]    (D on partitions, 8 chunks of 128), DMA'd in 4 S-quarters
  qT/kT [64h, S] per head (head dim on partitions), tiled in 4 chunks of 512
  v    [S, 64]   natural (keys on partitions) + ones column -> softmax denoms
  scoresT [k-block 128, q-window <=640] = kT_blk-stationary x qT-window
  exp (no max subtraction; scores are N(0,1)-ish), binary mask multiply,
  PV accumulates outT [65, 512-chunk] per head in rolling PSUM banks
  (row 64 = denominators), normalized per chunk as its last key-block lands,
  yT [D, S] = wo-stationary x outT_norm, staged bf16, summed on host.

Scheduling: emission order = Tile scheduler priority. The attention chain
(scores -> exp -> mask -> PV) is emitted first per pair; projection matmuls
(v, pair-1 q/k, y) are emitted after so the PE fills exp-paced dependency
stalls with them and the HAM clock gate never sees an idle window.
"""

import numpy as np
import ml_dtypes

bfloat16 = ml_dtypes.bfloat16

B, S, D = 2, 2048, 1024
H, Dh = 16, 64
NCORES = 8
NKB = S // 128  # 16 key blocks
WMAX = 640

_cache = {}


def _mask_rel():
    kp = np.arange(128)[:, None]
    j = np.arange(WMAX)[None, :]
    qrel = j - 256
    diff = np.abs(qrel - kp)
    seg = (qrel // 64) == (kp // 64)
    dil = (diff > 0) & (diff % 4 == 0) & (diff <= 256)
    return np.ascontiguousarray((seg | dil).astype(bfloat16))


def _win(kb):
    return max(0, kb - 2) * 128, min(NKB, kb + 3) * 128


def _pieces(q0, q1):
    """Split [q0,q1) at absolute 512 boundaries (psum bank + q-tile chunks)."""
    out = []
    a = q0
    while a < q1:
        b = min(q1, (a // 512 + 1) * 512)
        out.append((a, b))
        a = b
    return out


def _pieces_sc(q0, q1):
    """Split [q0,q1) at absolute 512 boundaries (q-tile chunks) AND at
    q0+512 (the [128,640] f32 score-psum tile's internal bank boundary)."""
    pts = {b for b in range((q0 // 512 + 1) * 512, q1, 512)}
    if q0 + 512 < q1:
        pts.add(q0 + 512)
    out = []
    a = q0
    for b in sorted(pts) + [q1]:
        if b > a:
            out.append((a, b))
            a = b
    return out


def _build(debug=False):
    key = ("nc", debug)
    if key in _cache:
        return _cache[key]
    import concourse.mybir as mybir
    from concourse import bacc
    from concourse.tile import TileContext

    bf = mybir.dt.bfloat16
    f32 = mybir.dt.float32
    EXP = mybir.ActivationFunctionType.Exp

    nc = bacc.Bacc()
    d_x = nc.declare_dram_parameter("xT", [128, 8, S], bf, isOutput=False)
    d_wq = nc.declare_dram_parameter("wq", [128, 8, 256], bf, isOutput=False)
    d_wk = nc.declare_dram_parameter("wk", [128, 8, 256], bf, isOutput=False)
    d_wv = nc.declare_dram_parameter("wv", [128, 8, 256], bf, isOutput=False)
    d_wo = nc.declare_dram_parameter("wo", [128, 2, 1024], bf, isOutput=False)
    d_mask = nc.declare_dram_parameter("maskT", [128, WMAX], bf, isOutput=False)
    d_y = nc.declare_dram_parameter("yT", [128, 8, S], bf, isOutput=True)
    if debug:
        d_dbg_q = nc.declare_dram_parameter("dbg_q", [128, S], bf, isOutput=True)
        d_dbg_k = nc.declare_dram_parameter("dbg_k", [128, S], bf, isOutput=True)
        d_dbg_v = nc.declare_dram_parameter("dbg_v", [128, 16 * 4 * 65], bf, isOutput=True)
        d_dbg_at = nc.declare_dram_parameter("dbg_at", [128, WMAX], bf, isOutput=True)
        d_dbg_on = nc.declare_dram_parameter("dbg_on", [128, S], bf, isOutput=True)
        d_dbg_den = nc.declare_dram_parameter("dbg_den", [4, S], f32, isOutput=True)
        d_dbg_rec = nc.declare_dram_parameter("dbg_rec", [4, S], f32, isOutput=True)

    with TileContext(nc) as tc:
        with (
            tc.tile_pool(name="const", bufs=1) as cpool,
            tc.tile_pool(name="attn", bufs=44) as apool,
            tc.tile_pool(name="ybig", bufs=2) as ypool,
            tc.tile_pool(name="small", bufs=4) as spool,
        ):
            # ---- input DMAs: weights+mask on scalar queue, x quarters split
            # across sync/gpsimd queues so the first quarter lands early ----
            sb_wq = cpool.tile([128, 8, 256], bf, name="wq", tag="wq")
            sb_wk = cpool.tile([128, 8, 256], bf, name="wk", tag="wk")
            nc.scalar.dma_start(out=sb_wq, in_=d_wq[:, :, :])
            nc.scalar.dma_start(out=sb_wk, in_=d_wk[:, :, :])
            sb_xq = []
            for sq in range(4):
                t = cpool.tile([128, 8, 512], bf, name=f"xq{sq}", tag=f"xq{sq}")
                nc.sync.dma_start(out=t[:, 0:4, :], in_=d_x[:, 0:4, sq * 512:(sq + 1) * 512])
                nc.gpsimd.dma_start(out=t[:, 4:8, :], in_=d_x[:, 4:8, sq * 512:(sq + 1) * 512])
                sb_xq.append(t)
            sb_mask = cpool.tile([128, WMAX], bf, name="mask", tag="mask")
            nc.scalar.dma_start(out=sb_mask, in_=d_mask[:, :])
            sb_wv = cpool.tile([128, 8, 256], bf, name="wv", tag="wv")
            nc.scalar.dma_start(out=sb_wv, in_=d_wv[:, :, :])
            sb_wo = cpool.tile([128, 2, 1024], bf, name="wo", tag="wo")
            nc.scalar.dma_start(out=sb_wo, in_=d_wo[:, :, :])

            sb_q = [[cpool.tile([128, 512], bf, name=f"q{p}{t}", tag=f"q{p}{t}")
                     for t in range(4)] for p in range(2)]
            sb_k = [[cpool.tile([128, 512], bf, name=f"k{p}{t}", tag=f"k{p}{t}")
                     for t in range(4)] for p in range(2)]
            sb_on = [[cpool.tile([128, 512], bf, name=f"on{p}{t}", tag=f"on{p}{t}")
                      for t in range(4)] for p in range(2)]
            sb_v = cpool.tile([128, 16, 4, 65], bf, name="v", tag="v")
            nc.vector.memset(sb_v[:, :, :, 64:65], 1.0)

            # ---- HAM warmup: junk matmuls lift the PE clock while DMAs land ----
            with tc.tile_pool(name="warm", bufs=1, space="PSUM") as wpool:
                junk = cpool.tile([128, 512], bf, name="junk", tag="junk")
                nc.vector.memset(junk, 0.0)
                wps = wpool.tile([128, 512], f32, name="wps", tag="wps")
                for i in range(0):
                    nc.tensor.matmul(wps, lhsT=junk[:, 0:128], rhs=junk,
                                     start=(i == 0), stop=(i == 0))

            # ---- main era: pj(2) + sc(4) + ot(2) psum banks ----
            with (
                tc.tile_pool(name="pj", bufs=2, space="PSUM") as pj,
                tc.tile_pool(name="sc", bufs=2, space="PSUM") as scp,
                tc.tile_pool(name="ot", bufs=2, space="PSUM") as otp,
            ):
                def proj_qk(w_sb, dst, p, tts, copy_eng, scope):
                    with nc.named_scope(scope):
                        for tt in tts:
                            ps = pj.tile([128, 512], f32, name="pspj", tag="pj")
                            for dc in range(8):
                                nc.tensor.matmul(
                                    ps,
                                    lhsT=w_sb[:, dc, p * 128:(p + 1) * 128],
                                    rhs=sb_xq[tt][:, dc, :],
                                    start=(dc == 0),
                                    stop=(dc == 7),
                                )
                            if copy_eng == "scalar":
                                nc.scalar.copy(dst[p][tt], ps)
                            else:
                                nc.vector.tensor_copy(dst[p][tt], ps)

                def proj_v(ts):
                    with nc.named_scope("proj_v"):
                        for t in ts:
                            ps = pj.tile([128, 256], f32, name="psv", tag="pj")
                            for dc in range(8):
                                nc.tensor.matmul(
                                    ps,
                                    lhsT=sb_xq[t // 4][:, dc, (t % 4) * 128:(t % 4) * 128 + 128],
                                    rhs=sb_wv[:, dc, :],
                                    start=(dc == 0),
                                    stop=(dc == 7),
                                )
                            src = ps.rearrange("p (h d) -> p h d", h=4)
                            if t % 2 == 0:
                                nc.scalar.copy(sb_v[:, t, :, 0:64], src)
                            else:
                                nc.vector.tensor_copy(sb_v[:, t, :, 0:64], src)

                def scores_kb(p, kb, ats):
                    q0, q1 = _win(kb)
                    wk_ = q1 - q0
                    j0 = q0 - (kb - 2) * 128
                    kbt, kbo = kb // 4, (kb % 4) * 128
                    for hh in range(2):
                        half = hh * 64
                        sc = scp.tile([128, WMAX], f32, name="sc", tag="sc")
                        for a, b in _pieces_sc(q0, q1):
                            nc.tensor.matmul(
                                sc[:, a - q0:b - q0],
                                lhsT=sb_k[p][kbt][half:half + 64, kbo:kbo + 128],
                                rhs=sb_q[p][a // 512][half:half + 64, a % 512:a % 512 + b - a],
                                start=True,
                                stop=True,
                            )
                        at = apool.tile([128, WMAX], bf, name="at", tag="at")
                        nc.scalar.activation(at[:, :wk_], sc[:, :wk_], EXP)
                        nc.vector.tensor_mul(
                            at[:, :wk_], at[:, :wk_], sb_mask[:, j0:j0 + wk_]
                        )
                        ats[p, hh, kb] = at
                        if debug and p == 0 and hh == 0 and kb == 8:
                            nc.sync.dma_start(out=d_dbg_at[:, :], in_=at[:, :])

                def pv_kb(p, hh, kb, ats, outc):
                    h = 2 * p + hh
                    half = hh * 64
                    q0, q1 = _win(kb)
                    at = ats[p, hh, kb]
                    vv = sb_v[:, kb, h, :]
                    for a, b in _pieces(q0, q1):
                        c = a // 512
                        if c not in outc:
                            outc[c] = otp.tile([65, 512], f32, name=f"o{h}{c}", tag="ot")
                        nc.tensor.matmul(
                            outc[c][:, a - 512 * c:b - 512 * c],
                            lhsT=vv,
                            rhs=at[:, a - q0:b - q0],
                            start=(kb == max(0, 4 * c - 2)),
                            stop=(kb == min(NKB - 1, 4 * c + 5)),
                        )
                    for c in sorted(outc):
                        if kb == min(NKB - 1, 4 * c + 5):
                            ot = outc.pop(c)
                            den = spool.tile([1, 512], f32, name="den", tag="den")
                            nc.vector.tensor_copy(den, ot[64:65, :])
                            rec = spool.tile([1, 512], f32, name="rec", tag="rec")
                            nc.vector.reciprocal_approx_fast(rec, den)
                            bc = spool.tile([64, 512], f32, name="bc", tag="bc")
                            nc.gpsimd.partition_broadcast(bc, rec)
                            if debug and p == 0:
                                nc.sync.dma_start(out=d_dbg_den[h:h + 1, c * 512:(c + 1) * 512], in_=den)
                                nc.sync.dma_start(out=d_dbg_rec[h:h + 1, c * 512:(c + 1) * 512], in_=rec)
                            nc.vector.tensor_mul(
                                sb_on[p][c][half:half + 64, :],
                                ot[0:64, :],
                                bc,
                            )

                def proj_y():
                    with nc.named_scope("proj_y"):
                        for tt in range(4):
                            yb = ypool.tile([128, 8, 512], bf, name=f"yb{tt}", tag="ybig")
                            for dc in range(8):
                                ps = pj.tile([128, 512], f32, name="psy", tag="pj")
                                nc.tensor.matmul(
                                    ps, lhsT=sb_wo[:, 0, dc * 128:(dc + 1) * 128],
                                    rhs=sb_on[0][tt], start=True, stop=False,
                                )
                                nc.tensor.matmul(
                                    ps, lhsT=sb_wo[:, 1, dc * 128:(dc + 1) * 128],
                                    rhs=sb_on[1][tt], start=False, stop=True,
                                )
                                if dc % 2 == 0:
                                    nc.scalar.copy(yb[:, dc, :], ps)
                                else:
                                    nc.vector.tensor_copy(yb[:, dc, :], ps)
                            eng = nc.sync if tt % 2 == 0 else nc.gpsimd
                            eng.dma_start(out=d_y[:, :, tt * 512:(tt + 1) * 512], in_=yb)

                # Attention chains emitted FIRST (= highest scheduler priority):
                # both pairs interleaved per kb so the exp stream is continuous.
                # Projections are emitted after -- the PE prefers ready attention
                # matmuls and fills every stall with projection work.
                ats = {}
                with nc.named_scope("scores"):
                    for kb in range(NKB):
                        scores_kb(0, kb, ats)
                        scores_kb(1, kb, ats)
                for p, hh in ((0, 0), (1, 0), (0, 1), (1, 1)):
                    with nc.named_scope(f"pv_h{2 * p + hh}"):
                        outc = {}
                        for kb in range(NKB):
                            pv_kb(p, hh, kb, ats, outc)
                # projections: filler priority, gated by x-quarter DMAs
                for sq in range(4):
                    proj_qk(sb_wq, sb_q, 0, [sq], "scalar", "proj_q0")
                    proj_qk(sb_wk, sb_k, 0, [sq], "scalar", "proj_k0")
                    proj_qk(sb_wq, sb_q, 1, [sq], "scalar", "proj_q1")
                    proj_qk(sb_wk, sb_k, 1, [sq], "scalar", "proj_k1")
                    proj_v(range(4 * sq, 4 * sq + 4))
                if debug:
                    for t in range(4):
                        nc.sync.dma_start(out=d_dbg_q[:, t * 512:(t + 1) * 512], in_=sb_q[0][t])
                        nc.sync.dma_start(out=d_dbg_k[:, t * 512:(t + 1) * 512], in_=sb_k[0][t])
                        nc.sync.dma_start(out=d_dbg_on[:, t * 512:(t + 1) * 512], in_=sb_on[0][t])
                    nc.sync.dma_start(
                        out=d_dbg_v[:, :],
                        in_=sb_v.rearrange("p a b c -> p (a b c)"),
                    )
                proj_y()

    nc.compile()
    _cache[key] = nc
    return nc


def kernel(hidden_states, w_q, w_k, w_v, w_o, _debug=False):
    from concourse.bass_utils import run_bass_kernel_spmd

    nc = _build(debug=_debug)
    mask = _mask_rel()
    scale = np.float32(Dh ** -0.5)

    def chunk_dmajor(w, rows, cols):
        return np.ascontiguousarray(
            w.reshape(rows, 128, cols).transpose(1, 0, 2)
        )

    in_maps = []
    for c in range(NCORES):
        b, hg = c // 4, c % 4
        hsl = slice(hg * 256, (hg + 1) * 256)
        xT = np.asarray(hidden_states[b]).T.astype(bfloat16)  # [D, S]
        in_maps.append({
            "xT": chunk_dmajor(xT, 8, S),
            "wq": chunk_dmajor((np.asarray(w_q[:, hsl]) * scale).astype(bfloat16), 8, 256),
            "wk": chunk_dmajor(np.asarray(w_k[:, hsl]).astype(bfloat16), 8, 256),
            "wv": chunk_dmajor(np.asarray(w_v[:, hsl]).astype(bfloat16), 8, 256),
            "wo": chunk_dmajor(np.asarray(w_o[hsl, :]).astype(bfloat16), 2, 1024),
            "maskT": mask,
        })

    res = run_bass_kernel_spmd(nc, in_maps, list(range(NCORES)))
    _cache["last_results"] = res

    y = np.zeros((B, S, D), np.float32)
    for c in range(NCORES):
        yT = np.asarray(res.results[c]["yT"], np.float32)  # [128, 8, S]
        y[c // 4] += yT.transpose(1, 0, 2).reshape(D, S).T
    return y
